# revision 57
# baseline (speedup 1.0000x reference)
"""Trainium2 Bass kernel for BasicCNN_LSTM (3x conv3x3+relu -> BN -> GAP -> LSTM -> BN -> dense).

Sharding: data-parallel over batch across 8 NeuronCores (4 batches/core).

Per-core plan (128 frames = 4 batches x 32 timesteps, processed as 64 frame-pairs):
  - conv1 (C=1 -> 48): host-built im2col [19, N] matmul (row 18 = ones for bias).
  - conv2/conv3 (48 -> 48): 9 tap-accumulated bf16 matmuls, 4 concurrent PE
    quadrants; activations in a split layout: pixel-row-half 0 at SBUF
    partitions 0:48, half 1 at partitions 64:112, each padded [16, 30].
    Partition rows 48 / 112 are constant 1.0; tap-0 weights carry the bias
    there, so the psum output is bias-included preactivation.
  - relu via ACT/DVE during psum->sbuf evacuation (no bias add needed).
  - GAP via one DVE scalar_tensor_tensor pass per (bank, frame): relu in place
    on psum with accum_out producing the pooled sum directly.
  - BN1 + /784 folded into LSTM input weights; LSTM gates at 32-strided psum
    partitions (f@0:8, i@32:40, o@64:72, g@96:104), two matmuls per step
    (Wx@Z K=112 + [Wh;bias]@[H;1] K=9), sigmoid/tanh on ACT, elementwise on
    GpSimd. Gate biases ride the ones-row of H (row 8). BN2 + output dense
    folded into a final matmul over the stored h sequence.
"""

import sys

sys.path.insert(0, "/opt/trn_rl_repo")

import numpy as np
import ml_dtypes

_BF16 = ml_dtypes.bfloat16

_NCORES = 8
_B, _T, _HW, _F, _U = 32, 32, 28, 48, 8
_EPS = 1e-3
_BPC = _B // _NCORES          # batches per core (4)
_PAIRS = _BPC * _T // 2       # frame pairs per core (64)

_F32 = np.float32


# ---------------------------------------------------------------------------
# Device program
# ---------------------------------------------------------------------------

def _build_program():
    import concourse.bass as bass  # noqa: F401
    import concourse.tile as tile
    from concourse.tile import add_dep_helper
    from concourse import bacc, mybir

    f32 = mybir.dt.float32
    bf16 = mybir.dt.bfloat16
    AF = mybir.ActivationFunctionType
    ALU = mybir.AluOpType
    AX = mybir.AxisListType

    nc = bacc.Bacc("TRN2", target_bir_lowering=False, debug=False, num_devices=_NCORES)

    # DRAM I/O
    x1_d = nc.dram_tensor("x1", (_PAIRS, 2, 19, 2, 196), bf16, kind="ExternalInput")
    w1_d = nc.dram_tensor("w1t", (128, 128), bf16, kind="ExternalInput")
    w2_d = nc.dram_tensor("w2t", (128, 9, 64), bf16, kind="ExternalInput")
    w3_d = nc.dram_tensor("w3t", (128, 9, 64), bf16, kind="ExternalInput")
    wx_d = nc.dram_tensor("wxt", (128, 128), f32, kind="ExternalInput")
    wh_d = nc.dram_tensor("wht", (9, 128), f32, kind="ExternalInput")
    bo_d = nc.dram_tensor("bot", (1, 1), f32, kind="ExternalInput")
    h0_d = nc.dram_tensor("h0", (9, 4), f32, kind="ExternalInput")
    ones_d = nc.dram_tensor("ones", (1, 2, 16, 30), bf16, kind="ExternalInput")
    wo_d = nc.dram_tensor("woutt", (128, 1), f32, kind="ExternalInput")
    out_d = nc.dram_tensor("out", (1, 128), f32, kind="ExternalOutput")

    from contextlib import ExitStack
    with tile.TileContext(nc) as tc, ExitStack() as ctx:
        # ---- persistent tiles (one const pool, unique tag per tile) ----
        cp = ctx.enter_context(tc.tile_pool(name="const", bufs=1))
        W1T = cp.tile([128, 128], bf16, name="W1T", tag="W1T")
        W2T = cp.tile([128, 9, 64], bf16, name="W2T", tag="W2T")
        W3T = cp.tile([128, 9, 64], bf16, name="W3T", tag="W3T")
        WXT = cp.tile([128, 128], f32, name="WXT", tag="WXT")
        WHT = cp.tile([9, 128], f32, name="WHT", tag="WHT")
        BOT = cp.tile([1, 1], f32, name="BOT", tag="BOT")
        ZERO = cp.tile([128, 1], f32, name="ZERO", tag="ZERO")
        WOT = cp.tile([128, 1], f32, name="WOT", tag="WOT")
        HST = cp.tile([128, 128], f32, name="HST", tag="HST")   # rows 0:8 h, col = 4t+b
        ZT = [cp.tile([128, 4], f32, name=f"ZT{i}", tag=f"ZT{i}") for i in range(3)]
        CT = [cp.tile([8, 4], f32, name=f"CT{i}", tag=f"CT{i}") for i in range(2)]
        HT = [cp.tile([9, 4], f32, name=f"HT{i}", tag=f"HT{i}") for i in range(2)]
        # persistent activation buffers (3-deep rotation per conv stage)
        A1B = [cp.tile([128, 2, 16, 30], bf16, name=f"A1B{i}", tag=f"A1B{i}")
               for i in range(3)]
        A2B = [cp.tile([128, 2, 16, 30], bf16, name=f"A2B{i}", tag=f"A2B{i}")
               for i in range(3)]

        nc.sync.dma_start(W1T[:, :], w1_d.ap()[:, :])
        nc.gpsimd.dma_start(W2T[:, :, :], w2_d.ap()[:, :, :])
        nc.scalar.dma_start(W3T[:, :, :], w3_d.ap()[:, :, :])
        nc.gpsimd.dma_start(WXT[:, :], wx_d.ap()[:, :])
        nc.gpsimd.dma_start(WHT[:, :], wh_d.ap()[:, :])
        nc.scalar.dma_start(WOT[:, :], wo_d.ap()[:, :])
        nc.scalar.dma_start(BOT[:, :], bo_d.ap()[:, :])

        # init state: c=0, h=0 (plus ones row 8 of H for the gate-bias trick;
        # DMA'd because engine memsets need 16-aligned partition bases)
        nc.vector.memset(CT[0][:, :], 0.0)
        nc.vector.memset(ZERO[:, :], 0.0)
        nc.sync.dma_start(HT[0][:, :], h0_d.ap()[:, :])
        nc.sync.dma_start(HT[1][:, :], h0_d.ap()[:, :])
        # activation buffers: zero everything (pads + unused rows), then the
        # constant-one bias rows at partitions 48 / 112 (DMA'd: engine memsets
        # require 32-aligned partition bases)
        for A in A1B + A2B:
            nc.gpsimd.memset(A[:, :, :, :], 0.0)
            nc.gpsimd.dma_start(A[48:49, :, :, :], ones_d.ap()[:, :, :, :])
            nc.gpsimd.dma_start(A[112:113, :, :, :], ones_d.ap()[:, :, :, :])

        # ---- pools ----
        x1_pool = ctx.enter_context(tc.tile_pool(name="x1p", bufs=8))
        ps_pool = ctx.enter_context(tc.tile_pool(name="psp", bufs=7, space="PSUM"))
        g_pool = ctx.enter_context(tc.tile_pool(name="gp", bufs=1, space="PSUM"))
        pc_pool = ctx.enter_context(tc.tile_pool(name="pcp", bufs=12))
        ls_pool = ctx.enter_context(tc.tile_pool(name="lsp", bufs=8))

        # PE warm-up: dummy full-width matmuls over the zeroed act buffers
        # keep the HAM activity window busy while the first input DMAs land,
        # so the first real convs run at 2.4 GHz instead of the cold 1.2.
        WUf = g_pool.tile([128, 512], f32, name="WUf", tag="gps")
        for _ in range(12):
            nc.tensor.matmul(WUf[0:60, 0:480], lhsT=A1B[0][:, 0, 0:2, 0:30],
                             rhs=A1B[0][:, :, 0:8, 0:30], skip_group_check=True)

        # per-pair state carried between loop stages
        X1 = [None] * _PAIRS
        P1 = [None] * _PAIRS   # (PA, PB, mA, mB) conv1
        P2 = [None] * _PAIRS
        P3 = [None] * _PAIRS

        def conv1(p):
            # block-diagonal K=19/M=128 packing: each streamed column carries
            # the 9 im2col taps (+ ones row for bias) of TWO output pixels
            # (chunk rows 0..6 at lhsT rows 0:9 -> psum 0:48, chunk rows 7..13
            # at rows 9:18 -> psum 64:112, bias row 18 -> both).
            X = x1_pool.tile([128, 2, 196], bf16, name="X")
            nc.sync.dma_start(X[0:19, :, :], x1_d.ap()[p, 0])
            nc.sync.dma_start(X[64:83, :, :], x1_d.ap()[p, 1])
            X1[p] = X
            PAf = ps_pool.tile([128, 512], f32, name="PAf", tag="cps")
            PBf = ps_pool.tile([128, 512], f32, name="PBf", tag="cps")
            PA, PB = PAf[:, 0:392], PBf[:, 0:392]
            mA = nc.tensor.matmul(PA[0:112, :], lhsT=W1T[0:19, 0:112], rhs=X[0:19, :, :],
                                  skip_group_check=True)
            mB = nc.tensor.matmul(PB[0:112, :], lhsT=W1T[64:83, 0:112], rhs=X[64:83, :, :],
                                  skip_group_check=True)
            P1[p] = (PA, PB, mA, mB)

        def conv23(p, W, A, Pout):
            PAf = ps_pool.tile([128, 512], f32, name="PA23f", tag="cps")
            PBf = ps_pool.tile([128, 512], f32, name="PB23f", tag="cps")
            PA, PB = PAf[:, 0:392], PBf[:, 0:392]
            for tap in range(9):
                dy, dx = tap // 3, tap % 3
                st = tap == 0
                sp = tap == 8
                k = 49 if tap == 0 else 48   # tap 0 carries the bias ones-row
                lo = W[0:k, tap, :]
                hi = W[64:64 + k, tap, :]
                nc.tensor.matmul(PA[0:64, :], lhsT=lo,
                                 rhs=A[0:k, :, dy:dy + 7, dx:dx + 28],
                                 start=st, stop=sp, skip_group_check=True)
                nc.tensor.matmul(PB[0:64, :], lhsT=hi,
                                 rhs=A[64:64 + k, :, dy:dy + 7, dx:dx + 28],
                                 start=st, stop=sp, skip_group_check=True)
                mA = nc.tensor.matmul(PA[64:128, :], lhsT=lo,
                                 rhs=A[0:k, :, 7 + dy:14 + dy, dx:dx + 28],
                                 start=st, stop=sp, skip_group_check=True)
                mB = nc.tensor.matmul(PB[64:128, :], lhsT=hi,
                                 rhs=A[64:64 + k, :, 7 + dy:14 + dy, dx:dx + 28],
                                 start=st, stop=sp, skip_group_check=True)
            Pout[p] = (PA, PB, mA, mB)

        def relu_store(p, Psrc, Adst_bufs, Aout, i3_on_act=False):
            # psum -> padded split act buffer, relu only (bias already in psum).
            PA, PB, mA, mB = Psrc[p]
            A = Adst_bufs[p % 3]
            # aligned: s00 -> half0 rows 1..7 ; s11 -> half1 rows 8..14
            i1 = nc.scalar.activation(A[0:48, :, 1:8, 1:29], PA[0:48, :], AF.Relu)
            i2 = nc.vector.tensor_scalar(A[64:112, :, 8:15, 1:29], PB[64:112, :],
                                         0.0, None, ALU.max)
            # crossed: s01 -> half0 rows 8..14 ; s10 -> half1 rows 1..7
            if i3_on_act:
                i3 = nc.scalar.activation(A[0:48, :, 8:15, 1:29], PA[64:112, :],
                                          AF.Relu)
            else:
                i3 = nc.vector.tensor_scalar(A[0:48, :, 8:15, 1:29], PA[64:112, :],
                                             0.0, None, ALU.max)
            i4 = nc.scalar.activation(A[64:112, :, 1:8, 1:29], PB[0:48, :], AF.Relu)
            # seam slivers: out row 13 -> half1 buf row 0 ; out row 14 -> half0 buf row 15
            pa3 = PA.rearrange("p (f r c) -> p f r c", f=2, r=7, c=28)
            pb3 = PB.rearrange("p (f r c) -> p f r c", f=2, r=7, c=28)
            i5 = nc.vector.tensor_scalar(A[64:112, :, 0:1, 1:29], pa3[64:112, :, 6:7, :],
                                         0.0, None, ALU.max)
            i6 = nc.scalar.activation(A[0:48, :, 15:16, 1:29], pb3[0:48, :, 0:1, :],
                                      AF.Relu)
            # PE-W + engine-R same-bank hazard: order every reader after the
            # tile's last matmul (PE completes in program order).
            for rd in (i1, i3, i5):
                add_dep_helper(rd.ins, mA.ins, reason="psum bank PA fully written")
            for rd in (i2, i4, i6):
                add_dep_helper(rd.ins, mB.ins, reason="psum bank PB fully written")
            Aout[p] = A

        def pool3(p):
            # GAP of relu(conv3): per (bank, frame), relu in place on psum
            # with accum_out giving the pooled sum in one DVE pass.
            # The /784 is folded into the LSTM input weights.
            PA, PB, mA, mB = P3[p]
            t, j = p // 2, p % 2
            Z = ZT[t % 3]
            pa = PA.rearrange("p (f n) -> p f n", f=2, n=196)
            pb = PB.rearrange("p (f n) -> p f n", f=2, n=196)
            acc = [pc_pool.tile([128, 1], f32, name=f"acc{i}") for i in range(4)]
            zb = ZERO[0:112, :].broadcast_to((112, 196))
            for fi in range(2):
                # relu-in-place + pooled sum in one DVE pass:
                # out = max(x, 0) + 0; accum_out = sum(out)
                r1 = nc.vector.scalar_tensor_tensor(
                    pa[0:112, fi, :], pa[0:112, fi, :], 0.0, zb,
                    ALU.max, ALU.add, accum_out=acc[2 * fi][0:112, :])
                r2 = nc.vector.scalar_tensor_tensor(
                    pb[0:112, fi, :], pb[0:112, fi, :], 0.0, zb,
                    ALU.max, ALU.add, accum_out=acc[2 * fi + 1][0:112, :])
                add_dep_helper(r1.ins, mA.ins, reason="psum bank PA fully written")
                add_dep_helper(r2.ins, mB.ins, reason="psum bank PB fully written")
                nc.vector.tensor_add(Z[0:112, 2 * j + fi:2 * j + fi + 1],
                                     acc[2 * fi][0:112, :], acc[2 * fi + 1][0:112, :])

        def lstm_step(t):
            # gate layout on psum partitions: f@0:8, i@32:40, o@64:72, g@96:104
            # (32-strided so ACT reads have aligned partition bases); two
            # matmuls per step: Wx@Z (K=112) + Wh,bias@[H;1] (K=9).
            Z = ZT[t % 3]
            Hp, Hn = HT[t % 2], HT[(t + 1) % 2]
            Gf = g_pool.tile([128, 512], f32, name="Gf", tag="gps")
            G = Gf[:, 0:4]
            nc.tensor.matmul(G[0:104, :], lhsT=WXT[0:112, 0:104], rhs=Z[0:112, :],
                             start=True, stop=False, skip_group_check=True)
            last = nc.tensor.matmul(G[0:104, :], lhsT=WHT[0:9, 0:104], rhs=Hp[0:9, :],
                                    start=False, stop=True, skip_group_check=True)
            GF = ls_pool.tile([8, 4], f32, name="GF")
            GI = ls_pool.tile([8, 4], f32, name="GI")
            GO = ls_pool.tile([8, 4], f32, name="GO")
            GG = ls_pool.tile([8, 4], f32, name="GG")
            a1 = nc.scalar.activation(GF[:, :], G[0:8, :], AF.Sigmoid)
            a2 = nc.scalar.activation(GI[:, :], G[32:40, :], AF.Sigmoid)
            a3 = nc.scalar.activation(GO[:, :], G[64:72, :], AF.Sigmoid)
            a4 = nc.scalar.activation(GG[:, :], G[96:104, :], AF.Tanh)
            for a in (a1, a2, a3, a4):
                add_dep_helper(a.ins, last.ins, reason="psum bank G fully written")
            Cp, Cn = CT[t % 2], CT[(t + 1) % 2]
            T1 = ls_pool.tile([8, 4], f32, name="T1")
            T2 = ls_pool.tile([8, 4], f32, name="T2")
            nc.gpsimd.tensor_mul(T1[:, :], GF[:, :], Cp[:, :])
            nc.gpsimd.tensor_mul(T2[:, :], GI[:, :], GG[:, :])
            nc.gpsimd.tensor_add(Cn[:, :], T1[:, :], T2[:, :])
            TC = ls_pool.tile([8, 4], f32, name="TC")
            nc.scalar.activation(TC[:, :], Cn[:, :], AF.Tanh)
            nc.gpsimd.tensor_mul(Hn[0:8, :], GO[:, :], TC[:, :])
            nc.gpsimd.tensor_mul(HST[0:8, 4 * t:4 * t + 4], GO[:, :], TC[:, :])

        # ---- software-pipelined emission ----
        A1 = [None] * _PAIRS
        A2 = [None] * _PAIRS
        for p in range(_PAIRS + 3):
            if p < _PAIRS:
                conv1(p)
            if 1 <= p < _PAIRS + 1:
                q = p - 1
                relu_store(q, P1, A1B, A1, i3_on_act=True)
                conv23(q, W2T, A1[q], P2)
            if 2 <= p < _PAIRS + 2:
                q = p - 2
                relu_store(q, P2, A2B, A2)
                conv23(q, W3T, A2[q], P3)
            if 3 <= p < _PAIRS + 3:
                pool3(p - 3)
            if p >= 4 and p % 2 == 0:
                lstm_step((p - 4) // 2)

        # ---- output head ----
        Yf = g_pool.tile([128, 512], f32, name="Yf", tag="gps")
        Y = Yf[0:1, 0:128]
        my = nc.tensor.matmul(Y[:, :], lhsT=WOT[0:8, :], rhs=HST[0:8, :])
        OUTS = cp.tile([1, 128], f32, name="OUTS", tag="OUTS")
        o1 = nc.vector.tensor_scalar(OUTS[:, :], Y[:, :], BOT[0:1, :], None, ALU.add)
        add_dep_helper(o1.ins, my.ins, reason="psum bank Y fully written")
        nc.sync.dma_start(out_d.ap()[:, :], OUTS[:, :])

    nc.compile()
    return nc


# ---------------------------------------------------------------------------
# Host-side prep
# ---------------------------------------------------------------------------

def _prep_core_inputs(xc, w1, b1, w2, b2, w3, b3, bn1, wf, bf, wi1, bi1, wi2, bi2,
                      wo, bo, bn2, w_out, b_out):
    """xc: [4, 32, 28, 28, 1] float32 for one core. Returns the in_map dict."""
    T, HW = _T, _HW
    xp = np.zeros((_BPC, T, 30, 30), _F32)
    xp[:, :, 1:29, 1:29] = xc[..., 0]

    # im2col for conv1, block-diagonal packed: rows 0:9 = taps of out rows
    # (h*14)+0..6, rows 9:18 = taps of out rows (h*14)+7..13, row 18 = ones
    X1 = np.empty((_PAIRS, 2, 19, 2, 196), _F32)
    X1v = X1.reshape(T, 2, 2, 19, 2, 196)  # [t, j, h, krow, fi, n]
    for h in range(2):
        for dy in range(3):
            for dx in range(3):
                blk = xp[:, :, h * 14 + dy:h * 14 + dy + 14, dx:dx + 28]  # [b, t, 14, 28]
                blk0 = blk[:, :, 0:7].reshape(_BPC, T, 196)
                blk1 = blk[:, :, 7:14].reshape(_BPC, T, 196)
                for j in range(2):
                    for fi in range(2):
                        X1v[:, j, h, 3 * dy + dx, fi] = blk0[2 * j + fi]
                        X1v[:, j, h, 9 + 3 * dy + dx, fi] = blk1[2 * j + fi]
    X1[:, :, 18] = 1.0

    def wpack1():
        # block diagonal: K rows 0:9 -> out cols 0:48, rows 9:18 -> cols 64:112
        # row 18 (ones) -> bias to both column groups
        w = np.zeros((128, 128), _F32)
        w9 = w1.reshape(9, _F)
        w[0:9, 0:48] = w9
        w[9:18, 64:112] = w9
        w[18, 0:48] = b1
        w[18, 64:112] = b1
        w[64:73, 0:48] = w9
        w[73:82, 64:112] = w9
        w[82, 0:48] = b1
        w[82, 64:112] = b1
        return w

    def wpack(wn, bn):
        w = np.zeros((128, 9, 64), _F32)
        for tap in range(9):
            m = wn[tap // 3, tap % 3]  # [48, 48]
            w[0:48, tap, 0:48] = m
            w[64:112, tap, 0:48] = m
        # bias rides the constant-one activation rows (48 / 112) on tap 0
        w[48, 0, 0:48] = bn
        w[112, 0, 0:48] = bn
        return w

    bn1_g, bn1_b, bn1_m, bn1_v = bn1
    bn2_g, bn2_b, bn2_m, bn2_v = bn2
    s1 = bn1_g / np.sqrt(bn1_v + _EPS)
    t1 = bn1_b - bn1_m * s1
    Wx = np.concatenate([wf[:_F], wi1[:_F], wo[:_F], wi2[:_F]], axis=1)  # [48, 32] f,i,o,g
    Wh = np.concatenate([wf[_F:], wi1[_F:], wo[_F:], wi2[_F:]], axis=1)  # [8, 32]
    bias = np.concatenate([bf, bi1, bo, bi2]) + t1 @ Wx                  # [32]
    # Z rows carry 784 * GAP-mean (relu sums over pixels)
    Wxs = (s1[:, None] * Wx) / float(HW * HW)

    def spread(m):
        # [r, 32] gate-packed (f,i,o,g x 8) -> [r, 128] at cols f@0:8, i@32:40,
        # o@64:72, g@96:104
        out = np.zeros((m.shape[0], 128), _F32)
        for gidx in range(4):
            out[:, 32 * gidx:32 * gidx + 8] = m[:, 8 * gidx:8 * gidx + 8]
        return out

    WX = np.zeros((128, 128), _F32)
    WX[0:48] = spread(Wxs)
    WX[64:112] = spread(Wxs)
    WH = np.zeros((9, 128), _F32)
    WH[0:8] = spread(Wh)
    WH[8] = spread(bias[None, :])[0]

    s2 = bn2_g / np.sqrt(bn2_v + _EPS)
    t2 = bn2_b - bn2_m * s2
    WO = np.zeros((128, 1), _F32)
    WO[0:8, 0] = s2 * w_out[:, 0]
    bot = np.array([[t2 @ w_out[:, 0] + b_out[0]]], _F32)

    return {
        "x1": X1.astype(_BF16),
        "w1t": wpack1().astype(_BF16),
        "w2t": wpack(w2, b2).astype(_BF16),
        "w3t": wpack(w3, b3).astype(_BF16),
        "wxt": WX, "wht": WH,
        "woutt": WO, "bot": bot,
        "h0": np.concatenate([np.zeros((8, 4), _F32), np.ones((1, 4), _F32)]),
        "ones": np.ones((1, 2, 16, 30), _BF16),
    }


_PROG = None
_LAST_RESULTS = None


def _install_ntff_hook():
    """The agent image's antenv lacks axon_hooks; synthesize it and register
    the ctypes-based NTFF profile hook from trn_agent_boot."""
    import types
    import antenv
    if getattr(antenv, "axon_hooks", None) is not None:
        return
    m = types.ModuleType("antenv.axon_hooks")
    state = {"h": None}
    m.set_axon_ntff_profile_hook = lambda h: state.__setitem__("h", h)
    m.get_axon_ntff_profile_hook = lambda: state["h"]
    sys.modules["antenv.axon_hooks"] = m
    antenv.axon_hooks = m
    try:
        from trn_agent_boot.trn_boot import _ntff_profile_via_ctypes
        m.set_axon_ntff_profile_hook(_ntff_profile_via_ctypes("/opt/axon/libaxon_pjrt.so"))
    except Exception as e:
        print("ntff hook install failed:", e)


def kernel(**inputs):
    global _PROG
    inp = {k: np.asarray(v, dtype=np.asarray(v).dtype) for k, v in inputs.items()}
    x = np.asarray(inp["x"], _F32)
    w2 = np.asarray(inp["w2"], _F32)
    w3 = np.asarray(inp["w3"], _F32)
    bn1 = tuple(np.asarray(inp[k], _F32) for k in ("bn1_g", "bn1_b", "bn1_m", "bn1_v"))
    bn2 = tuple(np.asarray(inp[k], _F32) for k in ("bn2_g", "bn2_b", "bn2_m", "bn2_v"))

    in_maps = []
    for c in range(_NCORES):
        xc = x[c * _BPC:(c + 1) * _BPC]
        in_maps.append(_prep_core_inputs(
            xc, np.asarray(inp["w1"], _F32), np.asarray(inp["b1"], _F32),
            w2, np.asarray(inp["b2"], _F32), w3, np.asarray(inp["b3"], _F32),
            bn1,
            np.asarray(inp["wf"], _F32), np.asarray(inp["bf"], _F32),
            np.asarray(inp["wi1"], _F32), np.asarray(inp["bi1"], _F32),
            np.asarray(inp["wi2"], _F32), np.asarray(inp["bi2"], _F32),
            np.asarray(inp["wo"], _F32), np.asarray(inp["bo"], _F32),
            bn2, np.asarray(inp["w_out"], _F32), np.asarray(inp["b_out"], _F32),
        ))

    if _PROG is None:
        _PROG = _build_program()
    from concourse.bass_utils import run_bass_kernel_spmd
    import os as _os
    if _os.environ.get("TRN_KERNEL_TRACE"):
        _install_ntff_hook()
    res = run_bass_kernel_spmd(_PROG, in_maps, core_ids=list(range(_NCORES)),
                               trace=bool(_os.environ.get("TRN_KERNEL_TRACE")))
    global _LAST_RESULTS
    _LAST_RESULTS = res

    out = np.empty((_B, _T, 1), _F32)
    for c in range(_NCORES):
        yc = res.results[c]["out"].reshape(_T, _BPC).T  # [4, 32]
        out[c * _BPC:(c + 1) * _BPC, :, 0] = yc
    return out


if __name__ == "__main__":
    pass


# revision 58
# speedup vs baseline: 1.0263x; 1.0263x over previous
"""Trainium2 Bass kernel for BasicCNN_LSTM (3x conv3x3+relu -> BN -> GAP -> LSTM -> BN -> dense).

Sharding: data-parallel over batch across 8 NeuronCores (4 batches/core).

Per-core plan (128 frames = 4 batches x 32 timesteps, processed as 64 frame-pairs):
  - conv1 (C=1 -> 48): host-built im2col [19, N] matmul (row 18 = ones for bias).
  - conv2/conv3 (48 -> 48): 9 tap-accumulated bf16 matmuls, 4 concurrent PE
    quadrants; activations in a split layout: pixel-row-half 0 at SBUF
    partitions 0:48, half 1 at partitions 64:112, each padded [16, 30].
    Partition rows 48 / 112 are constant 1.0; tap-0 weights carry the bias
    there, so the psum output is bias-included preactivation.
  - relu via ACT/DVE during psum->sbuf evacuation (no bias add needed).
  - GAP via one DVE scalar_tensor_tensor pass per (bank, frame): relu in place
    on psum with accum_out producing the pooled sum directly.
  - BN1 + /784 folded into LSTM input weights; LSTM gates at 32-strided psum
    partitions (f@0:8, i@32:40, o@64:72, g@96:104), two matmuls per step
    (Wx@Z K=112 + [Wh;bias]@[H;1] K=9), sigmoid/tanh on ACT, elementwise on
    GpSimd. Gate biases ride the ones-row of H (row 8). BN2 + output dense
    folded into a final matmul over the stored h sequence.
"""

import sys

sys.path.insert(0, "/opt/trn_rl_repo")

import numpy as np
import ml_dtypes

_BF16 = ml_dtypes.bfloat16

_NCORES = 8
_B, _T, _HW, _F, _U = 32, 32, 28, 48, 8
_EPS = 1e-3
_BPC = _B // _NCORES          # batches per core (4)
_PAIRS = _BPC * _T // 2       # frame pairs per core (64)

_F32 = np.float32


# ---------------------------------------------------------------------------
# Device program
# ---------------------------------------------------------------------------

def _build_program():
    import concourse.bass as bass  # noqa: F401
    import concourse.tile as tile
    from concourse.tile import add_dep_helper
    from concourse import bacc, mybir

    f32 = mybir.dt.float32
    bf16 = mybir.dt.bfloat16
    AF = mybir.ActivationFunctionType
    ALU = mybir.AluOpType
    AX = mybir.AxisListType

    nc = bacc.Bacc("TRN2", target_bir_lowering=False, debug=False, num_devices=_NCORES)

    # DRAM I/O
    x1_d = nc.dram_tensor("x1", (_PAIRS, 2, 19, 2, 196), bf16, kind="ExternalInput")
    w1_d = nc.dram_tensor("w1t", (128, 128), bf16, kind="ExternalInput")
    w2_d = nc.dram_tensor("w2t", (128, 9, 64), bf16, kind="ExternalInput")
    w3_d = nc.dram_tensor("w3t", (128, 9, 64), bf16, kind="ExternalInput")
    wx_d = nc.dram_tensor("wxt", (128, 128), f32, kind="ExternalInput")
    wh_d = nc.dram_tensor("wht", (9, 128), f32, kind="ExternalInput")
    bo_d = nc.dram_tensor("bot", (1, 1), f32, kind="ExternalInput")
    h0_d = nc.dram_tensor("h0", (9, 4), f32, kind="ExternalInput")
    ones_d = nc.dram_tensor("ones", (1, 2, 16, 30), bf16, kind="ExternalInput")
    wo_d = nc.dram_tensor("woutt", (128, 1), f32, kind="ExternalInput")
    out_d = nc.dram_tensor("out", (1, 128), f32, kind="ExternalOutput")

    from contextlib import ExitStack
    with tile.TileContext(nc) as tc, ExitStack() as ctx:
        # ---- persistent tiles (one const pool, unique tag per tile) ----
        cp = ctx.enter_context(tc.tile_pool(name="const", bufs=1))
        W1T = cp.tile([128, 128], bf16, name="W1T", tag="W1T")
        W2T = cp.tile([128, 9, 64], bf16, name="W2T", tag="W2T")
        W3T = cp.tile([128, 9, 64], bf16, name="W3T", tag="W3T")
        WXT = cp.tile([128, 128], f32, name="WXT", tag="WXT")
        WHT = cp.tile([9, 128], f32, name="WHT", tag="WHT")
        BOT = cp.tile([1, 1], f32, name="BOT", tag="BOT")
        ZERO = cp.tile([128, 1], f32, name="ZERO", tag="ZERO")
        WOT = cp.tile([128, 1], f32, name="WOT", tag="WOT")
        HST = cp.tile([128, 128], f32, name="HST", tag="HST")   # rows 0:8 h, col = 4t+b
        ZT = [cp.tile([128, 4], f32, name=f"ZT{i}", tag=f"ZT{i}") for i in range(3)]
        CT = [cp.tile([8, 4], f32, name=f"CT{i}", tag=f"CT{i}") for i in range(2)]
        HT = [cp.tile([9, 4], f32, name=f"HT{i}", tag=f"HT{i}") for i in range(2)]
        # persistent activation buffers (3-deep rotation per conv stage)
        A1B = [cp.tile([128, 2, 16, 30], bf16, name=f"A1B{i}", tag=f"A1B{i}")
               for i in range(3)]
        A2B = [cp.tile([128, 2, 16, 30], bf16, name=f"A2B{i}", tag=f"A2B{i}")
               for i in range(3)]

        nc.sync.dma_start(W1T[:, :], w1_d.ap()[:, :])
        nc.gpsimd.dma_start(W2T[:, :, :], w2_d.ap()[:, :, :])
        nc.scalar.dma_start(W3T[:, :, :], w3_d.ap()[:, :, :])
        nc.gpsimd.dma_start(WXT[:, :], wx_d.ap()[:, :])
        nc.gpsimd.dma_start(WHT[:, :], wh_d.ap()[:, :])
        nc.scalar.dma_start(WOT[:, :], wo_d.ap()[:, :])
        nc.scalar.dma_start(BOT[:, :], bo_d.ap()[:, :])

        # init state: c=0, h=0 (plus ones row 8 of H for the gate-bias trick;
        # DMA'd because engine memsets need 16-aligned partition bases)
        nc.vector.memset(CT[0][:, :], 0.0)
        nc.vector.memset(ZERO[:, :], 0.0)
        nc.sync.dma_start(HT[0][:, :], h0_d.ap()[:, :])
        nc.sync.dma_start(HT[1][:, :], h0_d.ap()[:, :])
        # activation buffers: zero everything (pads + unused rows), then the
        # constant-one bias rows at partitions 48 / 112 (DMA'd: engine memsets
        # require 32-aligned partition bases)
        for A in A1B + A2B:
            nc.gpsimd.memset(A[:, :, :, :], 0.0)
            nc.gpsimd.dma_start(A[48:49, :, :, :], ones_d.ap()[:, :, :, :])
            nc.gpsimd.dma_start(A[112:113, :, :, :], ones_d.ap()[:, :, :, :])

        # ---- pools ----
        x1_pool = ctx.enter_context(tc.tile_pool(name="x1p", bufs=8))
        ps_pool = ctx.enter_context(tc.tile_pool(name="psp", bufs=7, space="PSUM"))
        g_pool = ctx.enter_context(tc.tile_pool(name="gp", bufs=1, space="PSUM"))
        pc_pool = ctx.enter_context(tc.tile_pool(name="pcp", bufs=12))
        ls_pool = ctx.enter_context(tc.tile_pool(name="lsp", bufs=8))

        # PE warm-up: dummy full-width matmuls over the zeroed act buffers
        # keep the HAM activity window busy while the first input DMAs land,
        # so the first real convs run at 2.4 GHz instead of the cold 1.2.
        WUf = g_pool.tile([128, 512], f32, name="WUf", tag="gps")
        for _ in range(12):
            nc.tensor.matmul(WUf[0:60, 0:480], lhsT=A1B[0][:, 0, 0:2, 0:30],
                             rhs=A1B[0][:, :, 0:8, 0:30], skip_group_check=True)

        # per-pair state carried between loop stages
        X1 = [None] * _PAIRS
        P1 = [None] * _PAIRS   # (PA, PB, mA, mB) conv1
        P2 = [None] * _PAIRS
        P3 = [None] * _PAIRS

        def conv1(p):
            # block-diagonal K=19/M=128 packing: each streamed column carries
            # the 9 im2col taps (+ ones row for bias) of TWO output pixels
            # (chunk rows 0..6 at lhsT rows 0:9 -> psum 0:48, chunk rows 7..13
            # at rows 9:18 -> psum 64:112, bias row 18 -> both).
            X = x1_pool.tile([128, 2, 196], bf16, name="X")
            nc.sync.dma_start(X[0:19, :, :], x1_d.ap()[p, 0])
            nc.sync.dma_start(X[64:83, :, :], x1_d.ap()[p, 1])
            X1[p] = X
            PAf = ps_pool.tile([128, 512], f32, name="PAf", tag="cps")
            PBf = ps_pool.tile([128, 512], f32, name="PBf", tag="cps")
            PA, PB = PAf[:, 0:392], PBf[:, 0:392]
            mA = nc.tensor.matmul(PA[0:112, :], lhsT=W1T[0:19, 0:112], rhs=X[0:19, :, :],
                                  skip_group_check=True)
            mB = nc.tensor.matmul(PB[0:112, :], lhsT=W1T[64:83, 0:112], rhs=X[64:83, :, :],
                                  skip_group_check=True)
            P1[p] = (PA, PB, mA, mB)

        def conv23(p, W, A, Pout):
            PAf = ps_pool.tile([128, 512], f32, name="PA23f", tag="cps")
            PBf = ps_pool.tile([128, 512], f32, name="PB23f", tag="cps")
            PA, PB = PAf[:, 0:392], PBf[:, 0:392]
            for tap in range(9):
                dy, dx = tap // 3, tap % 3
                st = tap == 0
                sp = tap == 8
                k = 49 if tap == 0 else 48   # tap 0 carries the bias ones-row
                lo = W[0:k, tap, :]
                hi = W[64:64 + k, tap, :]
                nc.tensor.matmul(PA[0:64, :], lhsT=lo,
                                 rhs=A[0:k, :, dy:dy + 7, dx:dx + 28],
                                 start=st, stop=sp, skip_group_check=True)
                nc.tensor.matmul(PB[0:64, :], lhsT=hi,
                                 rhs=A[64:64 + k, :, dy:dy + 7, dx:dx + 28],
                                 start=st, stop=sp, skip_group_check=True)
                mA = nc.tensor.matmul(PA[64:128, :], lhsT=lo,
                                 rhs=A[0:k, :, 7 + dy:14 + dy, dx:dx + 28],
                                 start=st, stop=sp, skip_group_check=True)
                mB = nc.tensor.matmul(PB[64:128, :], lhsT=hi,
                                 rhs=A[64:64 + k, :, 7 + dy:14 + dy, dx:dx + 28],
                                 start=st, stop=sp, skip_group_check=True)
            Pout[p] = (PA, PB, mA, mB)

        def relu_store(p, Psrc, Adst_bufs, Aout, i3_on_act=False):
            # psum -> padded split act buffer, relu only (bias already in psum).
            PA, PB, mA, mB = Psrc[p]
            A = Adst_bufs[p % 3]
            # aligned: s00 -> half0 rows 1..7 ; s11 -> half1 rows 8..14
            i1 = nc.scalar.activation(A[0:48, :, 1:8, 1:29], PA[0:48, :], AF.Relu)
            i2 = nc.vector.tensor_scalar(A[64:112, :, 8:15, 1:29], PB[64:112, :],
                                         0.0, None, ALU.max)
            # crossed: s01 -> half0 rows 8..14 ; s10 -> half1 rows 1..7
            if i3_on_act:
                i3 = nc.scalar.activation(A[0:48, :, 8:15, 1:29], PA[64:112, :],
                                          AF.Relu)
            else:
                i3 = nc.vector.tensor_scalar(A[0:48, :, 8:15, 1:29], PA[64:112, :],
                                             0.0, None, ALU.max)
            i4 = nc.scalar.activation(A[64:112, :, 1:8, 1:29], PB[0:48, :], AF.Relu)
            # seam slivers: out row 13 -> half1 buf row 0 ; out row 14 -> half0 buf row 15
            pa3 = PA.rearrange("p (f r c) -> p f r c", f=2, r=7, c=28)
            pb3 = PB.rearrange("p (f r c) -> p f r c", f=2, r=7, c=28)
            i5 = nc.vector.tensor_scalar(A[64:112, :, 0:1, 1:29], pa3[64:112, :, 6:7, :],
                                         0.0, None, ALU.max)
            i6 = nc.scalar.activation(A[0:48, :, 15:16, 1:29], pb3[0:48, :, 0:1, :],
                                      AF.Relu)
            # PE-W + engine-R same-bank hazard: order every reader after the
            # tile's last matmul (PE completes in program order).
            for rd in (i1, i3, i5):
                add_dep_helper(rd.ins, mA.ins, reason="psum bank PA fully written")
            for rd in (i2, i4, i6):
                add_dep_helper(rd.ins, mB.ins, reason="psum bank PB fully written")
            Aout[p] = A

        def pool3(p):
            # GAP of relu(conv3): per (bank, frame), relu in place on psum
            # with accum_out giving the pooled sum in one DVE pass.
            # The /784 is folded into the LSTM input weights.
            PA, PB, mA, mB = P3[p]
            t, j = p // 2, p % 2
            Z = ZT[t % 3]
            pa = PA.rearrange("p (f n) -> p f n", f=2, n=196)
            pb = PB.rearrange("p (f n) -> p f n", f=2, n=196)
            acc = [pc_pool.tile([128, 1], f32, name=f"acc{i}") for i in range(4)]
            zb = ZERO[0:112, :].broadcast_to((112, 196))
            for fi in range(2):
                # relu-in-place + pooled sum in one DVE pass:
                # out = max(x, 0) + 0; accum_out = sum(out)
                r1 = nc.vector.scalar_tensor_tensor(
                    pa[0:112, fi, :], pa[0:112, fi, :], 0.0, zb,
                    ALU.max, ALU.add, accum_out=acc[2 * fi][0:112, :])
                r2 = nc.vector.scalar_tensor_tensor(
                    pb[0:112, fi, :], pb[0:112, fi, :], 0.0, zb,
                    ALU.max, ALU.add, accum_out=acc[2 * fi + 1][0:112, :])
                add_dep_helper(r1.ins, mA.ins, reason="psum bank PA fully written")
                add_dep_helper(r2.ins, mB.ins, reason="psum bank PB fully written")
                nc.vector.tensor_add(Z[0:112, 2 * j + fi:2 * j + fi + 1],
                                     acc[2 * fi][0:112, :], acc[2 * fi + 1][0:112, :])

        def lstm_step(t):
            # gate layout on psum partitions: f@0:8, i@32:40, o@64:72, g@96:104
            # (32-strided so ACT reads have aligned partition bases); two
            # matmuls per step: Wx@Z (K=112) + Wh,bias@[H;1] (K=9).
            Z = ZT[t % 3]
            Hp, Hn = HT[t % 2], HT[(t + 1) % 2]
            Gf = g_pool.tile([128, 512], f32, name="Gf", tag="gps")
            G = Gf[:, 0:4]
            nc.tensor.matmul(G[0:104, :], lhsT=WXT[0:112, 0:104], rhs=Z[0:112, :],
                             start=True, stop=False, skip_group_check=True)
            last = nc.tensor.matmul(G[0:104, :], lhsT=WHT[0:9, 0:104], rhs=Hp[0:9, :],
                                    start=False, stop=True, skip_group_check=True)
            GF = ls_pool.tile([8, 4], f32, name="GF")
            GI = ls_pool.tile([8, 4], f32, name="GI")
            GO = ls_pool.tile([8, 4], f32, name="GO")
            GG = ls_pool.tile([8, 4], f32, name="GG")
            a1 = nc.scalar.activation(GF[:, :], G[0:8, :], AF.Sigmoid)
            a2 = nc.scalar.activation(GI[:, :], G[32:40, :], AF.Sigmoid)
            a3 = nc.scalar.activation(GO[:, :], G[64:72, :], AF.Sigmoid)
            a4 = nc.scalar.activation(GG[:, :], G[96:104, :], AF.Tanh)
            for a in (a1, a2, a3, a4):
                add_dep_helper(a.ins, last.ins, reason="psum bank G fully written")
            Cp, Cn = CT[t % 2], CT[(t + 1) % 2]
            T1 = ls_pool.tile([8, 4], f32, name="T1")
            T2 = ls_pool.tile([8, 4], f32, name="T2")
            nc.gpsimd.tensor_mul(T1[:, :], GF[:, :], Cp[:, :])
            nc.gpsimd.tensor_mul(T2[:, :], GI[:, :], GG[:, :])
            nc.gpsimd.tensor_add(Cn[:, :], T1[:, :], T2[:, :])
            TC = ls_pool.tile([8, 4], f32, name="TC")
            nc.scalar.activation(TC[:, :], Cn[:, :], AF.Tanh)
            nc.gpsimd.tensor_mul(Hn[0:8, :], GO[:, :], TC[:, :])
            nc.gpsimd.tensor_mul(HST[0:8, 4 * t:4 * t + 4], GO[:, :], TC[:, :])

        # ---- software-pipelined emission ----
        A1 = [None] * _PAIRS
        A2 = [None] * _PAIRS
        for p in range(_PAIRS + 3):
            if p < _PAIRS:
                conv1(p)
            if 1 <= p < _PAIRS + 1:
                q = p - 1
                relu_store(q, P1, A1B, A1)
                conv23(q, W2T, A1[q], P2)
            if 2 <= p < _PAIRS + 2:
                q = p - 2
                relu_store(q, P2, A2B, A2)
                conv23(q, W3T, A2[q], P3)
            if 3 <= p < _PAIRS + 3:
                pool3(p - 3)
            if p >= 4 and p % 2 == 0:
                lstm_step((p - 4) // 2)

        # ---- output head ----
        Yf = g_pool.tile([128, 512], f32, name="Yf", tag="gps")
        Y = Yf[0:1, 0:128]
        my = nc.tensor.matmul(Y[:, :], lhsT=WOT[0:8, :], rhs=HST[0:8, :])
        OUTS = cp.tile([1, 128], f32, name="OUTS", tag="OUTS")
        o1 = nc.vector.tensor_scalar(OUTS[:, :], Y[:, :], BOT[0:1, :], None, ALU.add)
        add_dep_helper(o1.ins, my.ins, reason="psum bank Y fully written")
        nc.sync.dma_start(out_d.ap()[:, :], OUTS[:, :])

    nc.compile()
    return nc


# ---------------------------------------------------------------------------
# Host-side prep
# ---------------------------------------------------------------------------

def _prep_core_inputs(xc, w1, b1, w2, b2, w3, b3, bn1, wf, bf, wi1, bi1, wi2, bi2,
                      wo, bo, bn2, w_out, b_out):
    """xc: [4, 32, 28, 28, 1] float32 for one core. Returns the in_map dict."""
    T, HW = _T, _HW
    xp = np.zeros((_BPC, T, 30, 30), _F32)
    xp[:, :, 1:29, 1:29] = xc[..., 0]

    # im2col for conv1, block-diagonal packed: rows 0:9 = taps of out rows
    # (h*14)+0..6, rows 9:18 = taps of out rows (h*14)+7..13, row 18 = ones
    X1 = np.empty((_PAIRS, 2, 19, 2, 196), _F32)
    X1v = X1.reshape(T, 2, 2, 19, 2, 196)  # [t, j, h, krow, fi, n]
    for h in range(2):
        for dy in range(3):
            for dx in range(3):
                blk = xp[:, :, h * 14 + dy:h * 14 + dy + 14, dx:dx + 28]  # [b, t, 14, 28]
                blk0 = blk[:, :, 0:7].reshape(_BPC, T, 196)
                blk1 = blk[:, :, 7:14].reshape(_BPC, T, 196)
                for j in range(2):
                    for fi in range(2):
                        X1v[:, j, h, 3 * dy + dx, fi] = blk0[2 * j + fi]
                        X1v[:, j, h, 9 + 3 * dy + dx, fi] = blk1[2 * j + fi]
    X1[:, :, 18] = 1.0

    def wpack1():
        # block diagonal: K rows 0:9 -> out cols 0:48, rows 9:18 -> cols 64:112
        # row 18 (ones) -> bias to both column groups
        w = np.zeros((128, 128), _F32)
        w9 = w1.reshape(9, _F)
        w[0:9, 0:48] = w9
        w[9:18, 64:112] = w9
        w[18, 0:48] = b1
        w[18, 64:112] = b1
        w[64:73, 0:48] = w9
        w[73:82, 64:112] = w9
        w[82, 0:48] = b1
        w[82, 64:112] = b1
        return w

    def wpack(wn, bn):
        w = np.zeros((128, 9, 64), _F32)
        for tap in range(9):
            m = wn[tap // 3, tap % 3]  # [48, 48]
            w[0:48, tap, 0:48] = m
            w[64:112, tap, 0:48] = m
        # bias rides the constant-one activation rows (48 / 112) on tap 0
        w[48, 0, 0:48] = bn
        w[112, 0, 0:48] = bn
        return w

    bn1_g, bn1_b, bn1_m, bn1_v = bn1
    bn2_g, bn2_b, bn2_m, bn2_v = bn2
    s1 = bn1_g / np.sqrt(bn1_v + _EPS)
    t1 = bn1_b - bn1_m * s1
    Wx = np.concatenate([wf[:_F], wi1[:_F], wo[:_F], wi2[:_F]], axis=1)  # [48, 32] f,i,o,g
    Wh = np.concatenate([wf[_F:], wi1[_F:], wo[_F:], wi2[_F:]], axis=1)  # [8, 32]
    bias = np.concatenate([bf, bi1, bo, bi2]) + t1 @ Wx                  # [32]
    # Z rows carry 784 * GAP-mean (relu sums over pixels)
    Wxs = (s1[:, None] * Wx) / float(HW * HW)

    def spread(m):
        # [r, 32] gate-packed (f,i,o,g x 8) -> [r, 128] at cols f@0:8, i@32:40,
        # o@64:72, g@96:104
        out = np.zeros((m.shape[0], 128), _F32)
        for gidx in range(4):
            out[:, 32 * gidx:32 * gidx + 8] = m[:, 8 * gidx:8 * gidx + 8]
        return out

    WX = np.zeros((128, 128), _F32)
    WX[0:48] = spread(Wxs)
    WX[64:112] = spread(Wxs)
    WH = np.zeros((9, 128), _F32)
    WH[0:8] = spread(Wh)
    WH[8] = spread(bias[None, :])[0]

    s2 = bn2_g / np.sqrt(bn2_v + _EPS)
    t2 = bn2_b - bn2_m * s2
    WO = np.zeros((128, 1), _F32)
    WO[0:8, 0] = s2 * w_out[:, 0]
    bot = np.array([[t2 @ w_out[:, 0] + b_out[0]]], _F32)

    return {
        "x1": X1.astype(_BF16),
        "w1t": wpack1().astype(_BF16),
        "w2t": wpack(w2, b2).astype(_BF16),
        "w3t": wpack(w3, b3).astype(_BF16),
        "wxt": WX, "wht": WH,
        "woutt": WO, "bot": bot,
        "h0": np.concatenate([np.zeros((8, 4), _F32), np.ones((1, 4), _F32)]),
        "ones": np.ones((1, 2, 16, 30), _BF16),
    }


_PROG = None
_LAST_RESULTS = None


def _install_ntff_hook():
    """The agent image's antenv lacks axon_hooks; synthesize it and register
    the ctypes-based NTFF profile hook from trn_agent_boot."""
    import types
    import antenv
    if getattr(antenv, "axon_hooks", None) is not None:
        return
    m = types.ModuleType("antenv.axon_hooks")
    state = {"h": None}
    m.set_axon_ntff_profile_hook = lambda h: state.__setitem__("h", h)
    m.get_axon_ntff_profile_hook = lambda: state["h"]
    sys.modules["antenv.axon_hooks"] = m
    antenv.axon_hooks = m
    try:
        from trn_agent_boot.trn_boot import _ntff_profile_via_ctypes
        m.set_axon_ntff_profile_hook(_ntff_profile_via_ctypes("/opt/axon/libaxon_pjrt.so"))
    except Exception as e:
        print("ntff hook install failed:", e)


def kernel(**inputs):
    global _PROG
    inp = {k: np.asarray(v, dtype=np.asarray(v).dtype) for k, v in inputs.items()}
    x = np.asarray(inp["x"], _F32)
    w2 = np.asarray(inp["w2"], _F32)
    w3 = np.asarray(inp["w3"], _F32)
    bn1 = tuple(np.asarray(inp[k], _F32) for k in ("bn1_g", "bn1_b", "bn1_m", "bn1_v"))
    bn2 = tuple(np.asarray(inp[k], _F32) for k in ("bn2_g", "bn2_b", "bn2_m", "bn2_v"))

    in_maps = []
    for c in range(_NCORES):
        xc = x[c * _BPC:(c + 1) * _BPC]
        in_maps.append(_prep_core_inputs(
            xc, np.asarray(inp["w1"], _F32), np.asarray(inp["b1"], _F32),
            w2, np.asarray(inp["b2"], _F32), w3, np.asarray(inp["b3"], _F32),
            bn1,
            np.asarray(inp["wf"], _F32), np.asarray(inp["bf"], _F32),
            np.asarray(inp["wi1"], _F32), np.asarray(inp["bi1"], _F32),
            np.asarray(inp["wi2"], _F32), np.asarray(inp["bi2"], _F32),
            np.asarray(inp["wo"], _F32), np.asarray(inp["bo"], _F32),
            bn2, np.asarray(inp["w_out"], _F32), np.asarray(inp["b_out"], _F32),
        ))

    if _PROG is None:
        _PROG = _build_program()
    from concourse.bass_utils import run_bass_kernel_spmd
    import os as _os
    if _os.environ.get("TRN_KERNEL_TRACE"):
        _install_ntff_hook()
    res = run_bass_kernel_spmd(_PROG, in_maps, core_ids=list(range(_NCORES)),
                               trace=bool(_os.environ.get("TRN_KERNEL_TRACE")))
    global _LAST_RESULTS
    _LAST_RESULTS = res

    out = np.empty((_B, _T, 1), _F32)
    for c in range(_NCORES):
        yc = res.results[c]["out"].reshape(_T, _BPC).T  # [4, 32]
        out[c * _BPC:(c + 1) * _BPC, :, 0] = yc
    return out


if __name__ == "__main__":
    pass
